# revision 4
# baseline (speedup 1.0000x reference)
"""Trainium2 Bass/Tile kernel for MAB-style attention block (nn_MAB_channel_aware_force).

Reference computation (per batch b of 32):
  q = Q @ Wq + bq ; k = K @ Wk + bk ; v = K @ Wv + bv          # [512, 512]
  per head h (8 heads, dh=64):
    scores = qh @ kh^T / sqrt(512) ; A = softmax(scores)
    oh = qh + A @ vh
  O = LN0(concat(oh)) ; O = O + relu(O @ Wo + bo) ; out = LN1(O)

Sharding: data-parallel over batch across 8 NeuronCores (4 batches/core).

Layout strategy per core:
  - inputs PE-transposed to feature-major (Q^T, K^T)
  - q^T, k^T computed feature-major (lhsT=W, rhs=X^T); v computed natural
    (lhsT=K^T, rhs=Wv); q natural recovered by transposing q^T
  - scores^T per head = (kh^T)^T-mm  (c=64, row-packed pairs via base_partition)
  - exp on ACT (bounded args -> no max subtraction); A@V with ones-augmented
    v (m=65) gives attn^T and softmax sums in one PSUM tile
  - PE-transpose attn^T/sums back to natural; normalize+residual on DVE
  - LN via bn_stats/bn_aggr; rsqrt = exp(-0.5*ln(var+eps)) keeps ACT on one
    function table (no table-switch stalls)
  - fc: lhsT = LN0-out^T (PE-transposed), rhs = Wo; relu on ACT; LN1; DMA out
"""

import numpy as np

import concourse.bass as bass
import concourse.mybir as mybir
import concourse.tile as tile
from concourse import bacc
from concourse.bass_utils import run_bass_kernel_spmd
from concourse.masks import make_identity

P = 128
S = 512          # sequence length (Sq == Sk)
D = 512          # model dim == DIM_Q == DIM_K == DIM_V
H = 8            # heads
DH = D // H      # 64
NB = 4           # batches per core
NCORES = 8
EPS = 1e-5
SC = 1.0 / float(np.sqrt(D))
F32 = mybir.dt.float32
BF16 = mybir.dt.bfloat16
AF = mybir.ActivationFunctionType
OP = mybir.AluOpType

NBLK = S // P    # 4 sequence blocks of 128
NDB = D // P     # 4 feature blocks of 128


def build_program(zero_bias: bool, unit_ln: bool):
    nc = bacc.Bacc("TRN2", target_bir_lowering=False, debug=False)

    Qd = nc.declare_dram_parameter("Q", [NB, S, D], F32, isOutput=False)
    Kd = nc.declare_dram_parameter("K", [NB, S, D], F32, isOutput=False)
    Wd = {}
    for w in ("Wq", "Wk", "Wv", "Wo"):
        Wd[w] = nc.declare_dram_parameter(w, [D, D], F32, isOutput=False)
    Bd = {}
    for v in ("bq", "bk", "bv", "bo", "ln0_g", "ln0_b", "ln1_g", "ln1_b"):
        Bd[v] = nc.declare_dram_parameter(v, [D], F32, isOutput=False)
    Od = nc.declare_dram_parameter("out", [NB, S, D], F32, isOutput=True)

    with tile.TileContext(nc) as tc:
        _build(nc, tc, Qd, Kd, Wd, Bd, Od, zero_bias, unit_ln)
    nc.compile()
    return nc


def _build(nc, tc, Qd, Kd, Wd, Bd, Od, zero_bias, unit_ln):
    from contextlib import ExitStack

    ctx = ExitStack()
    with ctx:
        const = ctx.enter_context(tc.tile_pool(name="const", bufs=1))
        stage = ctx.enter_context(tc.tile_pool(name="stage", bufs=2))
        loadp = ctx.enter_context(tc.tile_pool(name="loadp", bufs=6))
        n16p = ctx.enter_context(tc.tile_pool(name="n16p", bufs=10))
        t16p = ctx.enter_context(tc.tile_pool(name="t16p", bufs=10))
        projp = ctx.enter_context(tc.tile_pool(name="projp", bufs=10))
        vaugp = ctx.enter_context(tc.tile_pool(name="vaugp", bufs=5))
        qnatp = ctx.enter_context(tc.tile_pool(name="qnatp", bufs=5))
        expp = ctx.enter_context(tc.tile_pool(name="expp", bufs=10))
        atnp = ctx.enter_context(tc.tile_pool(name="atnp", bufs=5))
        r1p = ctx.enter_context(tc.tile_pool(name="r1p", bufs=4))
        rbp = ctx.enter_context(tc.tile_pool(name="rbp", bufs=4))
        ohp = ctx.enter_context(tc.tile_pool(name="ohp", bufs=5))
        ln0p = ctx.enter_context(tc.tile_pool(name="ln0p", bufs=5))
        lntp = ctx.enter_context(tc.tile_pool(name="lntp", bufs=5))
        relup = ctx.enter_context(tc.tile_pool(name="relup", bufs=5))
        outp = ctx.enter_context(tc.tile_pool(name="outp", bufs=5))
        statp = ctx.enter_context(tc.tile_pool(name="statp", bufs=10))

        # PSUM: 8 banks total. pp: matmul accumulation; pt: transpose staging
        # (shared tag, 1 bank slots) incl. attn-out tiles.
        pp = ctx.enter_context(tc.tile_pool(name="pp", bufs=4, space="PSUM"))
        pt = ctx.enter_context(tc.tile_pool(name="pt", bufs=4, space="PSUM"))

        # ---- one-time constants ----
        I128b = const.tile([P, P], BF16)
        make_identity(nc, I128b)
        epsT = const.tile([P, 1], F32)
        nc.vector.memset(epsT[:], EPS)

        W16 = {}
        for w in ("Wq", "Wk", "Wv", "Wo"):
            st = stage.tile([P, NDB, D], F32, tag="wstage")
            nc.sync.dma_start(st[:], Wd[w].ap().rearrange("(o p) n -> p o n", p=P))
            W16[w] = const.tile([P, NDB, D], BF16, tag=f"w16_{w}", name=f"w16_{w}")
            nc.vector.tensor_copy(W16[w][:], st[:])

        if not zero_bias:
            # feature-major per-partition biases for q^T / k^T ACT copyback
            bqT = const.tile([P, NDB], F32, tag="bqT")
            nc.sync.dma_start(bqT[:], Bd["bq"].ap().rearrange("(o p) -> p o", p=P))
            bkT = const.tile([P, NDB], F32, tag="bkT")
            nc.sync.dma_start(bkT[:], Bd["bk"].ap().rearrange("(o p) -> p o", p=P))
            # free-axis broadcast tiles for natural-layout bias adds
            bc = {}
            for v in ("bv", "bo"):
                st = stage.tile([1, D], F32, tag="vstage")
                nc.sync.dma_start(st[:], Bd[v].ap()[None, :])
                bc[v] = const.tile([P, D], F32, tag=f"bc_{v}", name=f"bc_{v}")
                nc.gpsimd.partition_broadcast(bc[v][:], st[:])
            bv_bc, bo_bc = bc["bv"], bc["bo"]
        if not unit_ln:
            gbc = {}
            for v in ("ln0_g", "ln0_b", "ln1_g", "ln1_b"):
                st = stage.tile([1, D], F32, tag="vstage")
                nc.sync.dma_start(st[:], Bd[v].ap()[None, :])
                gbc[v] = const.tile([P, D], F32, tag=f"bc_{v}", name=f"bc_{v}")
                nc.gpsimd.partition_broadcast(gbc[v][:], st[:])

        def layer_norm(src_f32, out_tile_pool, out_dtype, g_bc, b_bc, tag):
            """src [128, 512] f32 -> LN over free axis -> new tile."""
            st6 = statp.tile([P, 6], F32, tag="st6")
            nc.vector.bn_stats(st6[:], src_f32[:])
            mv = statp.tile([P, 2], F32, tag="mv")
            nc.vector.bn_aggr(mv[:], st6[:])
            lnv = statp.tile([P, 1], F32, tag="lnv")
            nc.scalar.activation(lnv[:], mv[:, 1:2], AF.Ln, bias=epsT[:])
            istd = statp.tile([P, 1], F32, tag="istd")
            nc.scalar.activation(istd[:], lnv[:], AF.Exp, scale=-0.5)
            out = out_tile_pool.tile([P, D], out_dtype, tag=tag, name="lnout")
            if g_bc is None:
                nc.vector.tensor_scalar(
                    out[:], src_f32[:], mv[:, 0:1], istd[:], OP.subtract, OP.mult
                )
            else:
                t = statp.tile([P, D], F32, tag="lntmp")
                nc.vector.tensor_scalar(
                    t[:], src_f32[:], mv[:, 0:1], istd[:], OP.subtract, OP.mult
                )
                t2 = statp.tile([P, D], F32, tag="lntmp2")
                nc.vector.tensor_tensor(t2[:], t[:], g_bc[:], OP.mult)
                nc.vector.tensor_tensor(out[:], t2[:], b_bc[:], OP.add)
            return out

        for b in range(NB):
            # ---- stage A: load + cast + transpose inputs ----
            XT16 = {}  # name -> list of NDB tiles [128(d), 512(s)] bf16
            for name, dram in (("Q", Qd), ("K", Kd)):
                n16 = []
                for si in range(NBLK):
                    ld = loadp.tile([P, D], F32, tag="ld")
                    nc.sync.dma_start(ld[:], dram[b, si * P:(si + 1) * P, :])
                    c16 = n16p.tile([P, D], BF16, tag="n16")
                    nc.gpsimd.tensor_copy(c16[:], ld[:])
                    n16.append(c16)
                tlist = []
                for dj in range(NDB):
                    ps = pt.tile([P, S], BF16, tag="pt")
                    for si in range(NBLK):
                        nc.tensor.transpose(
                            ps[:, si * P:(si + 1) * P],
                            n16[si][:, dj * P:(dj + 1) * P],
                            I128b[:],
                        )
                    t16 = t16p.tile([P, S], BF16, tag="t16")
                    nc.vector.tensor_copy(t16[:], ps[:])
                    tlist.append(t16)
                XT16[name] = tlist
            QT16, KT16 = XT16["Q"], XT16["K"]

            # ---- stage B: projections ----
            qT16, kT16 = [], []
            for dst, wname, bT, src in (
                (qT16, "Wq", None if zero_bias else bqT, QT16),
                (kT16, "Wk", None if zero_bias else bkT, KT16),
            ):
                for vi in range(NDB):
                    ps = pp.tile([P, S], F32, tag="pp")
                    for dj in range(NDB):
                        nc.tensor.matmul(
                            ps[:],
                            W16[wname][:, dj, vi * P:(vi + 1) * P],
                            src[dj][:],
                            start=(dj == 0),
                            stop=(dj == NDB - 1),
                        )
                    t = projp.tile([P, S], BF16, tag="projT")
                    if bT is None:
                        nc.scalar.activation(t[:], ps[:], AF.Copy)
                    else:
                        nc.scalar.activation(t[:], ps[:], AF.Identity, bias=bT[:, vi:vi + 1])
                    dst.append(t)

            # v natural, ones-augmented: [128, 8 heads, 65]
            v_aug = []
            for si in range(NBLK):
                ps = pp.tile([P, S], F32, tag="pp")
                for dj in range(NDB):
                    nc.tensor.matmul(
                        ps[:],
                        KT16[dj][:, si * P:(si + 1) * P],
                        W16["Wv"][:, dj, :],
                        start=(dj == 0),
                        stop=(dj == NDB - 1),
                    )
                va = vaugp.tile([P, H, DH + 1], BF16, tag="vaug")
                nc.vector.memset(va[:, :, DH:DH + 1], 1.0)
                if zero_bias:
                    nc.vector.tensor_copy(
                        va[:, :, 0:DH], ps.rearrange("p (h d) -> p h d", h=H)
                    )
                else:
                    nc.vector.tensor_tensor(
                        va[:, :, 0:DH],
                        ps.rearrange("p (h d) -> p h d", h=H),
                        bv_bc.rearrange("p (h d) -> p h d", h=H),
                        OP.add,
                    )
                v_aug.append(va)

            # q natural (bf16) via transpose of q^T
            q_nat = []
            for si in range(NBLK):
                ps = pt.tile([P, S], BF16, tag="pt")
                for vi in range(NDB):
                    nc.tensor.transpose(
                        ps[:, vi * P:(vi + 1) * P],
                        qT16[vi][:, si * P:(si + 1) * P],
                        I128b[:],
                    )
                qn = qnatp.tile([P, S], BF16, tag="qnat")
                nc.vector.tensor_copy(qn[:], ps[:])
                q_nat.append(qn)

            # ---- stage C: attention per head ----
            attnT = [atnp.tile([P, S], BF16, tag="atnT", name=f"atnT{i}") for i in range(NDB)]
            for h in range(H):
                vi = h // 2
                hof = (h % 2) * DH
                expA = []
                for ki in range(NBLK):
                    ps = pp.tile([P, S], F32, tag="pp")
                    nc.tensor.matmul(
                        ps[:],
                        kT16[vi][hof:hof + DH, ki * P:(ki + 1) * P],
                        qT16[vi][hof:hof + DH, :],
                        start=True,
                        stop=True,
                    )
                    ea = expp.tile([P, S], BF16, tag="expA")
                    nc.scalar.activation(ea[:], ps[:], AF.Exp, scale=SC)
                    expA.append(ea)
                po = pt.tile([P, S], F32, tag="pt")
                for ki in range(NBLK):
                    nc.tensor.matmul(
                        po[0:DH + 1, :],
                        v_aug[ki][:, h, :],
                        expA[ki][:],
                        start=(ki == 0),
                        stop=(ki == NBLK - 1),
                    )
                # normalize while copying back: attnT = attn_unnorm * (1/sums)
                r1 = r1p.tile([1, S], F32, tag="r1")
                nc.vector.reciprocal(r1[:], po[DH:DH + 1, :])
                rb = rbp.tile([DH, S], F32, tag="rb")
                nc.gpsimd.partition_broadcast(rb[:], r1[:])
                nc.vector.tensor_tensor(
                    attnT[vi][hof:hof + DH, :], po[0:DH, :], rb[:], OP.mult
                )

            # ---- stage D: back to natural; normalize + residual ----
            oh = []
            for si in range(NBLK):
                pa = pt.tile([P, S], BF16, tag="pt")
                for vi in range(NDB):
                    nc.tensor.transpose(
                        pa[:, vi * P:(vi + 1) * P],
                        attnT[vi][:, si * P:(si + 1) * P],
                        I128b[:],
                    )
                o = ohp.tile([P, D], F32, tag="oh")
                nc.vector.tensor_tensor(o[:], pa[:], q_nat[si][:], OP.add)
                oh.append(o)

            # ---- stage E: LN0 ----
            ln0 = []
            for si in range(NBLK):
                g = None if unit_ln else gbc["ln0_g"]
                bb = None if unit_ln else gbc["ln0_b"]
                ln0.append(layer_norm(oh[si], ln0p, BF16, g, bb, "ln0"))

            # ---- stage F: fc + relu + residual + LN1 + store ----
            lnT = []
            for vi in range(NDB):
                ps = pt.tile([P, S], BF16, tag="pt")
                for si in range(NBLK):
                    nc.tensor.transpose(
                        ps[:, si * P:(si + 1) * P],
                        ln0[si][:, vi * P:(vi + 1) * P],
                        I128b[:],
                    )
                t = lntp.tile([P, S], BF16, tag="lnT")
                nc.vector.tensor_copy(t[:], ps[:])
                lnT.append(t)

            for si in range(NBLK):
                ps = pp.tile([P, S], F32, tag="pp")
                for dj in range(NDB):
                    nc.tensor.matmul(
                        ps[:],
                        lnT[dj][:, si * P:(si + 1) * P],
                        W16["Wo"][:, dj, :],
                        start=(dj == 0),
                        stop=(dj == NDB - 1),
                    )
                rl = relup.tile([P, D], F32, tag="relu")
                if zero_bias:
                    nc.scalar.activation(rl[:], ps[:], AF.Relu)
                else:
                    tmp = relup.tile([P, D], F32, tag="fcb")
                    nc.vector.tensor_tensor(tmp[:], ps[:], bo_bc[:], OP.add)
                    nc.scalar.activation(rl[:], tmp[:], AF.Relu)
                pre2 = relup.tile([P, D], F32, tag="pre2")
                nc.vector.tensor_tensor(pre2[:], rl[:], ln0[si][:], OP.add)

                g = None if unit_ln else gbc["ln1_g"]
                bb = None if unit_ln else gbc["ln1_b"]
                of = layer_norm(pre2, outp, F32, g, bb, "outf")
                nc.sync.dma_start(Od[b, si * P:(si + 1) * P, :], of[:])


_CACHE = {}


def _get_program(zero_bias: bool, unit_ln: bool):
    key = (zero_bias, unit_ln)
    if key not in _CACHE:
        _CACHE[key] = build_program(zero_bias, unit_ln)
    return _CACHE[key]


def _make_in_maps(inputs):
    Q = np.ascontiguousarray(inputs["Q"], dtype=np.float32)
    K = np.ascontiguousarray(inputs["K"], dtype=np.float32)
    shared = {
        name: np.ascontiguousarray(inputs[name], dtype=np.float32)
        for name in ("Wq", "Wk", "Wv", "Wo", "bq", "bk", "bv", "bo",
                     "ln0_g", "ln0_b", "ln1_g", "ln1_b")
    }
    in_maps = []
    for c in range(NCORES):
        m = dict(shared)
        m["Q"] = Q[c * NB:(c + 1) * NB]
        m["K"] = K[c * NB:(c + 1) * NB]
        in_maps.append(m)
    return in_maps


def run(inputs, trace=False):
    zero_bias = all(
        not np.any(inputs[v]) for v in ("bq", "bk", "bv", "bo")
    )
    unit_ln = (
        np.all(inputs["ln0_g"] == 1.0) and np.all(inputs["ln1_g"] == 1.0)
        and not np.any(inputs["ln0_b"]) and not np.any(inputs["ln1_b"])
    )
    nc = _get_program(zero_bias, unit_ln)
    res = run_bass_kernel_spmd(
        nc, _make_in_maps(inputs), core_ids=list(range(NCORES)), trace=trace
    )
    out = np.concatenate([res.results[c]["out"] for c in range(NCORES)], axis=0)
    return out, res


def kernel(**inputs):
    B, Sq, Dq = inputs["Q"].shape
    assert (B, Sq, Dq) == (NB * NCORES, S, D), (B, Sq, Dq)
    out, _ = run(inputs, trace=False)
    return out


# revision 6
# speedup vs baseline: 1.3203x; 1.3203x over previous
"""Trainium2 Bass/Tile kernel for MAB-style attention block (nn_MAB_channel_aware_force).

Reference computation (per batch b of 32):
  q = Q @ Wq + bq ; k = K @ Wk + bk ; v = K @ Wv + bv          # [512, 512]
  per head h (8 heads, dh=64):
    scores = qh @ kh^T / sqrt(512) ; A = softmax(scores)
    oh = qh + A @ vh
  O = LN0(concat(oh)) ; O = O + relu(O @ Wo + bo) ; out = LN1(O)

Sharding: data-parallel over batch across 8 NeuronCores (4 batches/core).

Layout strategy per core:
  - inputs PE-transposed to feature-major (Q^T, K^T); all loads+casts hoisted
    to kernel start so batch boundaries don't stall PE
  - q^T, k^T computed feature-major (lhsT=W, rhs=X^T); v computed natural
    (lhsT=K^T, rhs=Wv); q natural recovered by transposing q^T
  - scores^T per head (c=64, row-pairs via base_partition 0/64); exp on ACT
    (bounded args -> no max subtraction); A@V with ones-augmented v (m=65)
    gives attn^T and softmax sums in one PSUM tile
  - sums row is DMAed PSUM->DRAM, then gathered DRAM->SBUF transposed to
    [s, h]; reciprocal on free=8 is cheap; normalization+residual applied in
    natural layout after PE-transposing attn^T back
  - LN via bn_stats/bn_aggr with Ln/Exp rsqrt batched to [128,4] and
    clustered to minimize ACT table switches
  - fc: lhsT = LN0-out^T (PE-transposed), rhs = Wo; relu on ACT; LN1; DMA out
"""

import numpy as np

import concourse.bass as bass
import concourse.mybir as mybir
import concourse.tile as tile
from concourse import bacc
from concourse.bass_utils import run_bass_kernel_spmd
from concourse.masks import make_identity

P = 128
S = 512          # sequence length (Sq == Sk)
D = 512          # model dim == DIM_Q == DIM_K == DIM_V
H = 8            # heads
DH = D // H      # 64
NB = 4           # batches per core
NCORES = 8
EPS = 1e-5
SC = 1.0 / float(np.sqrt(D))
F32 = mybir.dt.float32
BF16 = mybir.dt.bfloat16
AF = mybir.ActivationFunctionType
OP = mybir.AluOpType

NBLK = S // P    # 4 sequence blocks of 128
NDB = D // P     # 4 feature blocks of 128


def build_program(zero_bias: bool, unit_ln: bool):
    nc = bacc.Bacc("TRN2", target_bir_lowering=False, debug=False)

    Qd = nc.declare_dram_parameter("Q", [NB, S, D], F32, isOutput=False)
    Kd = nc.declare_dram_parameter("K", [NB, S, D], F32, isOutput=False)
    Wd = {}
    for w in ("Wq", "Wk", "Wv", "Wo"):
        Wd[w] = nc.declare_dram_parameter(w, [D, D], F32, isOutput=False)
    Bd = {}
    for v in ("bq", "bk", "bv", "bo", "ln0_g", "ln0_b", "ln1_g", "ln1_b"):
        Bd[v] = nc.declare_dram_parameter(v, [D], F32, isOutput=False)
    Od = nc.declare_dram_parameter("out", [NB, S, D], F32, isOutput=True)
    sums_d = nc.dram_tensor("sums_scratch", [NB, H, S], F32)

    with tile.TileContext(nc) as tc:
        _build(nc, tc, Qd, Kd, Wd, Bd, Od, sums_d, zero_bias, unit_ln)
    nc.compile()
    return nc


def _build(nc, tc, Qd, Kd, Wd, Bd, Od, sums_d, zero_bias, unit_ln):
    from contextlib import ExitStack

    ctx = ExitStack()
    with ctx:
        const = ctx.enter_context(tc.tile_pool(name="const", bufs=1))
        stage = ctx.enter_context(tc.tile_pool(name="stage", bufs=2))
        loadp = ctx.enter_context(tc.tile_pool(name="loadp", bufs=10))
        t16p = ctx.enter_context(tc.tile_pool(name="t16p", bufs=10))
        projp = ctx.enter_context(tc.tile_pool(name="projp", bufs=10))
        vaugp = ctx.enter_context(tc.tile_pool(name="vaugp", bufs=5))
        qnatp = ctx.enter_context(tc.tile_pool(name="qnatp", bufs=5))
        expp = ctx.enter_context(tc.tile_pool(name="expp", bufs=10))
        atnp = ctx.enter_context(tc.tile_pool(name="atnp", bufs=5))
        rnp = ctx.enter_context(tc.tile_pool(name="rnp", bufs=8))
        ohp = ctx.enter_context(tc.tile_pool(name="ohp", bufs=5))
        ln0p = ctx.enter_context(tc.tile_pool(name="ln0p", bufs=5))
        lntp = ctx.enter_context(tc.tile_pool(name="lntp", bufs=5))
        relup = ctx.enter_context(tc.tile_pool(name="relup", bufs=5))
        outp = ctx.enter_context(tc.tile_pool(name="outp", bufs=5))
        statp = ctx.enter_context(tc.tile_pool(name="statp", bufs=10))

        # PSUM: 8 banks. pp: matmul accumulation (proj/scores/fc);
        # pt: transpose staging + attn-out tiles.
        pp = ctx.enter_context(tc.tile_pool(name="pp", bufs=5, space="PSUM"))
        pt = ctx.enter_context(tc.tile_pool(name="pt", bufs=3, space="PSUM"))

        # ---- one-time constants ----
        I128b = const.tile([P, P], BF16)
        make_identity(nc, I128b)
        epsT = const.tile([P, 1], F32)
        nc.vector.memset(epsT[:], EPS)

        W16 = {}
        for w in ("Wq", "Wk", "Wv", "Wo"):
            st = stage.tile([P, NDB, D], F32, tag="wstage")
            nc.sync.dma_start(st[:], Wd[w].ap().rearrange("(o p) n -> p o n", p=P))
            W16[w] = const.tile([P, NDB, D], BF16, tag=f"w16_{w}", name=f"w16_{w}")
            nc.vector.tensor_copy(W16[w][:], st[:])

        if not zero_bias:
            bqT = const.tile([P, NDB], F32, tag="bqT")
            nc.sync.dma_start(bqT[:], Bd["bq"].ap().rearrange("(o p) -> p o", p=P))
            bkT = const.tile([P, NDB], F32, tag="bkT")
            nc.sync.dma_start(bkT[:], Bd["bk"].ap().rearrange("(o p) -> p o", p=P))
            bc = {}
            for v in ("bv", "bo"):
                st = stage.tile([1, D], F32, tag="vstage")
                nc.sync.dma_start(st[:], Bd[v].ap()[None, :])
                bc[v] = const.tile([P, D], F32, tag=f"bc_{v}", name=f"bc_{v}")
                nc.gpsimd.partition_broadcast(bc[v][:], st[:])
            bv_bc, bo_bc = bc["bv"], bc["bo"]
        if not unit_ln:
            gbc = {}
            for v in ("ln0_g", "ln0_b", "ln1_g", "ln1_b"):
                st = stage.tile([1, D], F32, tag="vstage")
                nc.sync.dma_start(st[:], Bd[v].ap()[None, :])
                gbc[v] = const.tile([P, D], F32, tag=f"bc_{v}", name=f"bc_{v}")
                nc.gpsimd.partition_broadcast(gbc[v][:], st[:])

        # ---- hoisted loads + bf16 casts for ALL batches ----
        QK16 = {}
        for name, dram in (("Q", Qd), ("K", Kd)):
            big = const.tile([P, NB * NBLK, D], BF16, tag=f"n16_{name}",
                             name=f"n16_{name}")
            for b in range(NB):
                for si in range(NBLK):
                    ld = loadp.tile([P, D], F32, tag="ld", name="ld")
                    nc.sync.dma_start(ld[:], dram[b, si * P:(si + 1) * P, :])
                    if name == "Q":
                        nc.scalar.activation(big[:, b * NBLK + si, :], ld[:], AF.Copy)
                    else:
                        nc.vector.tensor_copy(big[:, b * NBLK + si, :], ld[:])
            QK16[name] = big

        def ln_stats(srcs):
            """srcs: list of NBLK [128, 512] f32 tiles -> (mv4, istd4)."""
            mv4 = statp.tile([P, NBLK, 2], F32, tag="mv4", name="mv4")
            for si in range(NBLK):
                st6 = statp.tile([P, 6], F32, tag="st6", name="st6")
                nc.vector.bn_stats(st6[:], srcs[si][:])
                nc.vector.bn_aggr(mv4[:, si, :], st6[:])
            lnv = statp.tile([P, NBLK], F32, tag="lnv", name="lnv")
            nc.scalar.activation(lnv[:], mv4[:, :, 1], AF.Ln, bias=epsT[:])
            istd4 = statp.tile([P, NBLK], F32, tag="istd4", name="istd4")
            nc.scalar.activation(istd4[:], lnv[:], AF.Exp, scale=-0.5)
            return mv4, istd4

        def ln_apply(src, mv4, istd4, si, pool, dtype, g_bc, b_bc, tag):
            out = pool.tile([P, D], dtype, tag=tag, name="lnout")
            if g_bc is None:
                nc.vector.tensor_scalar(
                    out[:], src[:], mv4[:, si, 0:1], istd4[:, si:si + 1],
                    OP.subtract, OP.mult,
                )
            else:
                t = statp.tile([P, D], F32, tag="lntmp", name="lntmp")
                nc.vector.tensor_scalar(
                    t[:], src[:], mv4[:, si, 0:1], istd4[:, si:si + 1],
                    OP.subtract, OP.mult,
                )
                t2 = statp.tile([P, D], F32, tag="lntmp2", name="lntmp2")
                nc.vector.tensor_tensor(t2[:], t[:], g_bc[:], OP.mult)
                nc.vector.tensor_tensor(out[:], t2[:], b_bc[:], OP.add)
            return out

        for b in range(NB):
            Qn = [QK16["Q"][:, b * NBLK + si, :] for si in range(NBLK)]
            Kn = [QK16["K"][:, b * NBLK + si, :] for si in range(NBLK)]

            # ---- stage A: transpose inputs to feature-major ----
            XT16 = {}
            for name, n16 in (("Q", Qn), ("K", Kn)):
                tlist = []
                for dj in range(NDB):
                    ps = pt.tile([P, S], BF16, tag="pt")
                    for si in range(NBLK):
                        nc.tensor.transpose(
                            ps[:, si * P:(si + 1) * P],
                            n16[si][:, dj * P:(dj + 1) * P],
                            I128b[:],
                        )
                    t16 = t16p.tile([P, S], BF16, tag="t16")
                    nc.vector.tensor_copy(t16[:], ps[:])
                    tlist.append(t16)
                XT16[name] = tlist
            QT16, KT16 = XT16["Q"], XT16["K"]

            # ---- stage B: projections ----
            qT16, kT16 = [], []
            for dst, wname, bT, src in (
                (qT16, "Wq", None if zero_bias else bqT, QT16),
                (kT16, "Wk", None if zero_bias else bkT, KT16),
            ):
                for vi in range(NDB):
                    ps = pp.tile([P, S], F32, tag="pp")
                    for dj in range(NDB):
                        nc.tensor.matmul(
                            ps[:],
                            W16[wname][:, dj, vi * P:(vi + 1) * P],
                            src[dj][:],
                            start=(dj == 0),
                            stop=(dj == NDB - 1),
                        )
                    t = projp.tile([P, S], BF16, tag="projT")
                    if bT is None:
                        nc.scalar.activation(t[:], ps[:], AF.Copy)
                    else:
                        nc.scalar.activation(t[:], ps[:], AF.Identity, bias=bT[:, vi:vi + 1])
                    dst.append(t)

            # v natural, ones-augmented: [128, 8 heads, 65]
            v_aug = []
            for si in range(NBLK):
                ps = pp.tile([P, S], F32, tag="pp")
                for dj in range(NDB):
                    nc.tensor.matmul(
                        ps[:],
                        KT16[dj][:, si * P:(si + 1) * P],
                        W16["Wv"][:, dj, :],
                        start=(dj == 0),
                        stop=(dj == NDB - 1),
                    )
                va = vaugp.tile([P, H, DH + 1], BF16, tag="vaug")
                nc.vector.memset(va[:, :, DH:DH + 1], 1.0)
                if zero_bias:
                    nc.vector.tensor_copy(
                        va[:, :, 0:DH], ps.rearrange("p (h d) -> p h d", h=H)
                    )
                else:
                    nc.vector.tensor_tensor(
                        va[:, :, 0:DH],
                        ps.rearrange("p (h d) -> p h d", h=H),
                        bv_bc.rearrange("p (h d) -> p h d", h=H),
                        OP.add,
                    )
                v_aug.append(va)

            # q natural (bf16) via transpose of q^T
            q_nat = []
            for si in range(NBLK):
                ps = pt.tile([P, S], BF16, tag="pt")
                for vi in range(NDB):
                    nc.tensor.transpose(
                        ps[:, vi * P:(vi + 1) * P],
                        qT16[vi][:, si * P:(si + 1) * P],
                        I128b[:],
                    )
                qn = qnatp.tile([P, S], BF16, tag="qnat")
                nc.vector.tensor_copy(qn[:], ps[:])
                q_nat.append(qn)

            # ---- stage C: attention per head ----
            attnT = [atnp.tile([P, S], BF16, tag="atnT", name=f"atnT{i}")
                     for i in range(NDB)]
            for h in range(H):
                vi = h // 2
                hof = (h % 2) * DH
                expA = []
                for ki in range(NBLK):
                    ps = pp.tile([P, S], F32, tag="pp")
                    nc.tensor.matmul(
                        ps[:],
                        kT16[vi][hof:hof + DH, ki * P:(ki + 1) * P],
                        qT16[vi][hof:hof + DH, :],
                        start=True,
                        stop=True,
                    )
                    ea = expp.tile([P, S], BF16, tag="expA")
                    nc.scalar.activation(ea[:], ps[:], AF.Exp, scale=SC)
                    expA.append(ea)
                po = pt.tile([P, S], F32, tag="pt")
                for ki in range(NBLK):
                    nc.tensor.matmul(
                        po[0:DH + 1, :],
                        v_aug[ki][:, h, :],
                        expA[ki][:],
                        start=(ki == 0),
                        stop=(ki == NBLK - 1),
                    )
                nc.vector.tensor_copy(attnT[vi][hof:hof + DH, :], po[0:DH, :])
                # softmax sums -> SBUF -> DRAM scratch (compute engines
                # cannot scatter to partition h, and DMA cannot read PSUM;
                # the DRAM bounce lets DMA gather them transposed below)
                s1 = rnp.tile([1, S], F32, tag="s1", name="s1")
                nc.scalar.activation(s1[:], po[DH:DH + 1, :], AF.Copy)
                nc.sync.dma_start(sums_d[b, h][None, :], s1[:])

            # ---- stage D: natural layout; normalize + residual ----
            oh = []
            for si in range(NBLK):
                sg = rnp.tile([P, H], F32, tag="sg", name="sg")
                nc.sync.dma_start(
                    sg[:], sums_d[b, :, si * P:(si + 1) * P].rearrange("h s -> s h")
                )
                rn = rnp.tile([P, H], F32, tag="rn", name="rn")
                nc.vector.reciprocal(rn[:], sg[:])

                pa = pt.tile([P, S], BF16, tag="pt")
                for vi in range(NDB):
                    nc.tensor.transpose(
                        pa[:, vi * P:(vi + 1) * P],
                        attnT[vi][:, si * P:(si + 1) * P],
                        I128b[:],
                    )
                o = ohp.tile([P, D], F32, tag="oh")
                nc.vector.tensor_tensor(
                    o.rearrange("p (h d) -> p h d", h=H),
                    pa.rearrange("p (h d) -> p h d", h=H),
                    rn[:, :, None].to_broadcast((P, H, DH)),
                    OP.mult,
                )
                nc.vector.tensor_tensor(o[:], o[:], q_nat[si][:], OP.add)
                oh.append(o)

            # ---- stage E: LN0 (batched stats; Ln/Exp clustered) ----
            g0 = None if unit_ln else gbc["ln0_g"]
            b0 = None if unit_ln else gbc["ln0_b"]
            mv4, istd4 = ln_stats(oh)
            ln0 = [ln_apply(oh[si], mv4, istd4, si, ln0p, BF16, g0, b0, "ln0")
                   for si in range(NBLK)]

            # ---- stage F: fc + relu + residual + LN1 + store ----
            lnT = []
            for vi in range(NDB):
                ps = pt.tile([P, S], BF16, tag="pt")
                for si in range(NBLK):
                    nc.tensor.transpose(
                        ps[:, si * P:(si + 1) * P],
                        ln0[si][:, vi * P:(vi + 1) * P],
                        I128b[:],
                    )
                t = lntp.tile([P, S], BF16, tag="lnT")
                nc.vector.tensor_copy(t[:], ps[:])
                lnT.append(t)

            pre2 = []
            for si in range(NBLK):
                ps = pp.tile([P, S], F32, tag="pp")
                for dj in range(NDB):
                    nc.tensor.matmul(
                        ps[:],
                        lnT[dj][:, si * P:(si + 1) * P],
                        W16["Wo"][:, dj, :],
                        start=(dj == 0),
                        stop=(dj == NDB - 1),
                    )
                rl = relup.tile([P, D], F32, tag="relu")
                if zero_bias:
                    nc.scalar.activation(rl[:], ps[:], AF.Relu)
                else:
                    tmp = relup.tile([P, D], F32, tag="fcb")
                    nc.vector.tensor_tensor(tmp[:], ps[:], bo_bc[:], OP.add)
                    nc.scalar.activation(rl[:], tmp[:], AF.Relu)
                p2 = relup.tile([P, D], F32, tag="pre2")
                nc.vector.tensor_tensor(p2[:], rl[:], ln0[si][:], OP.add)
                pre2.append(p2)

            g1 = None if unit_ln else gbc["ln1_g"]
            b1 = None if unit_ln else gbc["ln1_b"]
            mv4b, istd4b = ln_stats(pre2)
            for si in range(NBLK):
                of = ln_apply(pre2[si], mv4b, istd4b, si, outp, F32, g1, b1, "outf")
                nc.sync.dma_start(Od[b, si * P:(si + 1) * P, :], of[:])


_CACHE = {}


def _get_program(zero_bias: bool, unit_ln: bool):
    key = (zero_bias, unit_ln)
    if key not in _CACHE:
        _CACHE[key] = build_program(zero_bias, unit_ln)
    return _CACHE[key]


def _make_in_maps(inputs):
    Q = np.ascontiguousarray(inputs["Q"], dtype=np.float32)
    K = np.ascontiguousarray(inputs["K"], dtype=np.float32)
    shared = {
        name: np.ascontiguousarray(inputs[name], dtype=np.float32)
        for name in ("Wq", "Wk", "Wv", "Wo", "bq", "bk", "bv", "bo",
                     "ln0_g", "ln0_b", "ln1_g", "ln1_b")
    }
    in_maps = []
    for c in range(NCORES):
        m = dict(shared)
        m["Q"] = Q[c * NB:(c + 1) * NB]
        m["K"] = K[c * NB:(c + 1) * NB]
        in_maps.append(m)
    return in_maps


def run(inputs, trace=False):
    zero_bias = all(
        not np.any(inputs[v]) for v in ("bq", "bk", "bv", "bo")
    )
    unit_ln = (
        np.all(inputs["ln0_g"] == 1.0) and np.all(inputs["ln1_g"] == 1.0)
        and not np.any(inputs["ln0_b"]) and not np.any(inputs["ln1_b"])
    )
    nc = _get_program(zero_bias, unit_ln)
    res = run_bass_kernel_spmd(
        nc, _make_in_maps(inputs), core_ids=list(range(NCORES)), trace=trace
    )
    out = np.concatenate([res.results[c]["out"] for c in range(NCORES)], axis=0)
    return out, res


def kernel(**inputs):
    B, Sq, Dq = inputs["Q"].shape
    assert (B, Sq, Dq) == (NB * NCORES, S, D), (B, Sq, Dq)
    out, _ = run(inputs, trace=False)
    return out
